# revision 12
# baseline (speedup 1.0000x reference)
"""Multihead attention (custom freq-bias) Trainium2 Bass kernel.

Full inputs -> shard across 8 NeuronCores -> SPMD bass kernel -> host combine.

Sharding: core c handles batch b = c//2 and head-half s = c%2 (8 of 16 heads).
Each core computes, for its batch/half:
    qT = (Wq_s^T @ x_q^T) * 1/8 + bq_s   laid out [512 ch, 2048 q]   (scale folded on host)
    kT = Wk_s^T @ x_k^T + bk_s           laid out [512 ch, 2048 k]
    v  = x_v @ Wv_s + bv_s               laid out [2048 k, 512 ch], bf16,
                                          with a ones column per head (denominator trick)
    per head h: scoresT[k, q] = kT_h^T-contracted matmul; exp fused with PSUM
    eviction on ScalarE with per-partition freq-bias; AV matmul with ones column
    produces unnormalized y^T and softmax denominators; deferred normalization;
    out_partial = y @ Wp_s + bp/2  -> host sums the two head-half partials.
"""

import numpy as np

import concourse.bass as bass
import concourse.tile as tile
from concourse import bacc, mybir
from concourse.bass import ts

F32 = mybir.dt.float32
F32R = mybir.dt.float32r
BF16 = mybir.dt.bfloat16
AF = mybir.ActivationFunctionType

B, N, C, H, D = 4, 2048, 1024, 16, 64
NCORES = 8
HC = C // 2          # 512 channels per core (8 heads x 64)
NH = HC // D         # 8 heads per core
NKT = N // 128       # 16 key tiles
VA_W = NH * (D + 1)  # v_aug width: 8 heads x 65 cols


def _r(ap):
    return ap.bitcast(F32R)


def kernel_body(ctx, tc, out, ins):
    """Emit the per-core kernel. out: [2048, 1024] f32 DRAM. ins: dict of DRAM APs."""
    nc = tc.nc
    xq, xk, xv = ins["xqt"], ins["xkt"], ins["xvt"]      # [1024, 2048] f32
    wq, wk, wv = ins["wq"], ins["wk"], ins["wv"]          # [1024, 512] f32
    wp = ins["wp"]                                        # [512, 1024] f32
    bq, bk, bv = ins["bq"], ins["bk"], ins["bv"]          # [1, 512] f32
    bp = ins["bp"]                                        # [1, 1024] f32
    freq = ins["freq"]                                    # [128, 16] f32

    singles = ctx.enter_context(tc.tile_pool(name="singles", bufs=1))

    # ---- persistent SBUF residents ----
    qT = [singles.tile([128, N], F32R, name=f"qT{m}") for m in range(4)]
    kT = [singles.tile([128, N], F32R, name=f"kT{m}") for m in range(4)]
    yT = [singles.tile([128, N], F32R, name=f"yT{m}") for m in range(4)]
    vaug = [singles.tile([128, VA_W], BF16, name=f"vaug{i}") for i in range(NKT)]
    wp_sb = [singles.tile([128, C], F32R, name=f"wp{i}") for i in range(4)]
    ones_pad = singles.tile([128, 512], F32R, name="ones_pad")
    bq_pad = singles.tile([128, 512], F32R, name="bq_pad")
    bk_pad = singles.tile([128, 512], F32R, name="bk_pad")
    bv_pad = singles.tile([128, 512], F32R, name="bv_pad")
    bp_pad = singles.tile([128, C], F32R, name="bp_pad")
    freq_sb = singles.tile([128, NKT], F32, name="freq_sb")
    den = singles.tile([32, 512], F32, name="den")
    recip = singles.tile([32, 512], F32, name="recip")
    den_dram = nc.dram_tensor("den_scratch", [32, 512], F32, kind="Internal").ap()

    # constants / bias staging (host-prepared padded tiles; f32r memset is illegal)
    nc.sync.dma_start(out=ones_pad, in_=ins["ones"])
    for pad, src in ((bq_pad, bq), (bk_pad, bk), (bv_pad, bv), (bp_pad, bp)):
        nc.sync.dma_start(out=pad, in_=src)
    nc.sync.dma_start(out=freq_sb, in_=freq)
    for i in range(NKT):
        nc.vector.memset(vaug[i], 1.0)
    for i in range(4):
        nc.sync.dma_start(out=wp_sb[i], in_=wp[ts(i, 128), :])

    # ================= Phase 1: projections =================
    with (
        tc.tile_pool(name="xpool", bufs=12) as xpool,
        tc.tile_pool(name="wpool", bufs=8) as wpool,
        tc.tile_pool(name="psum1", bufs=4, space="PSUM") as psum1,
    ):
        # --- qT / kT: out[ch, q] = W^T @ x^T, ch-major tiles ---
        for (w_dram, x_dram, b_pad, dstT) in (
            (wq, xq, bq_pad, qT),
            (wk, xk, bk_pad, kT),
        ):
            w_sb = []
            for k in range(8):
                t = wpool.tile([128, 512], F32R, tag="w", name=f"w{k}")
                nc.sync.dma_start(out=t, in_=w_dram[ts(k, 128), :])
                w_sb.append(t)
            for nq in range(4):
                x_sb = []
                for k in range(8):
                    t = xpool.tile([128, 512], F32R, tag="x", name=f"x{k}")
                    nc.sync.dma_start(out=t, in_=x_dram[ts(k, 128), ts(nq, 512)])
                    x_sb.append(t)
                for m in range(4):
                    ps = psum1.tile([128, 512], F32, tag="ps1", name="ps_qk")
                    nc.tensor.matmul(ps, _r(b_pad[:, ts(m, 128)]), _r(ones_pad),
                                     start=True, stop=False)
                    for k in range(8):
                        nc.tensor.matmul(ps, _r(w_sb[k][:, ts(m, 128)]), _r(x_sb[k]),
                                         start=False, stop=(k == 7))
                    nc.vector.tensor_copy(dstT[m][:, ts(nq, 512)], ps)

        # --- v: out[n, ch] = x @ Wv, scattered into v_aug (bf16, ones cols kept) ---
        wv_sb = []
        for k in range(8):
            t = wpool.tile([128, 512], F32R, tag="w", name=f"wv{k}")
            nc.sync.dma_start(out=t, in_=wv[ts(k, 128), :])
            wv_sb.append(t)
        for ntg in range(4):
            xv_sb = []
            for k in range(8):
                t = xpool.tile([128, 512], F32R, tag="x", name=f"xv{k}")
                nc.sync.dma_start(out=t, in_=xv[ts(k, 128), ts(ntg, 512)])
                xv_sb.append(t)
            for ntl in range(4):
                nt = ntg * 4 + ntl
                ps = psum1.tile([128, 512], F32, tag="ps1", name="ps_v")
                nc.tensor.matmul(ps, _r(ones_pad[:, 0:128]), _r(bv_pad),
                                 start=True, stop=False)
                for k in range(8):
                    nc.tensor.matmul(ps, _r(xv_sb[k][:, ts(ntl, 128)]), _r(wv_sb[k]),
                                     start=False, stop=(k == 7))
                va = vaug[nt].rearrange("p (h c) -> p h c", c=D + 1)
                nc.vector.tensor_copy(va[:, :, 0:D],
                                      ps.rearrange("p (h c) -> p h c", c=D))

    # ================= Phase 2: attention =================
    with (
        tc.tile_pool(name="epool", bufs=11) as epool,
        tc.tile_pool(name="dpool", bufs=4) as dpool,
        tc.tile_pool(name="psS", bufs=2, space="PSUM") as psS,
        tc.tile_pool(name="psA", bufs=1, space="PSUM") as psA_pool,
    ):
        for h in range(NH):
            pair, po = h // 2, (h % 2) * 64
            psA = [psA_pool.tile([128, 512], F32, tag=f"psA{qc}", name=f"psA{qc}")
                   for qc in range(4)]
            ets = [None] * NKT
            for half in range(2):
                for kt8 in range(8):
                    kt = half * 8 + kt8
                    et = epool.tile([128, N], BF16, tag="exp", name="et")
                    for qp in range(2):
                        pss = psS.tile([128, 1024], F32, tag="pss", name="pss")
                        for j in range(2):
                            qc = qp * 2 + j
                            nc.tensor.matmul(
                                pss[:, ts(j, 512)],
                                _r(kT[pair][po:po + 64, ts(kt, 128)]),
                                _r(qT[pair][po:po + 64, ts(qc, 512)]),
                                start=True, stop=True)
                        nc.scalar.activation(out=et[:, ts(qp, 1024)], in_=pss,
                                             func=AF.Exp,
                                             bias=freq_sb[:, kt:kt + 1], scale=1.0)
                    ets[kt] = et
                for qc in range(4):
                    for kt8 in range(8):
                        kt = half * 8 + kt8
                        nc.tensor.matmul(
                            psA[qc][0:D + 1, :],
                            vaug[kt][:, h * (D + 1):(h + 1) * (D + 1)],
                            ets[kt][:, ts(qc, 512)],
                            start=(kt == 0), stop=(kt == NKT - 1),
                            skip_group_check=True)
            for qc in range(4):
                nc.vector.tensor_copy(yT[pair][po:po + 64, ts(qc, 512)],
                                      psA[qc][0:D, :])
                dt = dpool.tile([1, 512], F32, tag="dt", name="dt")
                nc.vector.tensor_copy(dt, psA[qc][D:D + 1, :])
                r = h * 4 + qc
                nc.sync.dma_start(out=den_dram[r:r + 1, :], in_=dt)

    # ================= Phase 3: normalize + output projection =================
    nc.sync.dma_start(out=den, in_=den_dram)
    nc.vector.reciprocal(recip, den)
    recip_dram = nc.dram_tensor("recip_scratch", [32, 512], F32, kind="Internal").ap()
    nc.sync.dma_start(out=recip_dram, in_=recip)
    with (
        tc.tile_pool(name="rpool", bufs=4) as rpool,
        tc.tile_pool(name="opool", bufs=4) as opool,
        tc.tile_pool(name="psO", bufs=4, space="PSUM") as psO,
    ):
        for pair in range(4):
            for qc in range(4):
                r = rpool.tile([128, 512], F32, tag="rbc", name="rbc")
                for hh in range(2):
                    src = recip_dram[(pair * 2 + hh) * 4 + qc:(pair * 2 + hh) * 4 + qc + 1, :]
                    bcast = bass.AP(tensor=src.tensor, offset=src.offset,
                                    ap=[[0, 64]] + list(src.ap)[1:])
                    nc.sync.dma_start(out=r[hh * 64:(hh + 1) * 64, :], in_=bcast)
                ysl = yT[pair][:, ts(qc, 512)]
                nc.vector.tensor_mul(ysl, ysl, r)
        for m in range(16):
            for n2 in range(2):
                ps = psO.tile([128, 512], F32, tag="psO", name="psO")
                nc.tensor.matmul(ps, _r(ones_pad[:, 0:128]),
                                 _r(bp_pad[:, ts(n2, 512)]), start=True, stop=False)
                for kp in range(4):
                    nc.tensor.matmul(ps, _r(yT[kp][:, ts(m, 128)]),
                                     _r(wp_sb[kp][:, ts(n2, 512)]),
                                     start=False, stop=(kp == 3))
                ot = opool.tile([128, 512], F32, tag="ot", name="ot")
                nc.scalar.copy(ot, ps)
                nc.sync.dma_start(out=out[ts(m, 128), ts(n2, 512)], in_=ot)


INPUT_SPECS = {
    "xqt": ([C, N], F32R), "xkt": ([C, N], F32R), "xvt": ([C, N], F32R),
    "wq": ([C, HC], F32R), "wk": ([C, HC], F32R), "wv": ([C, HC], F32R),
    "bq": ([128, HC], F32R), "bk": ([128, HC], F32R), "bv": ([128, HC], F32R),
    "wp": ([HC, C], F32R), "bp": ([128, C], F32R),
    "ones": ([128, 512], F32R),
    "freq": ([128, NKT], F32),
}


def build_nc():
    from contextlib import ExitStack
    nc = bacc.Bacc("TRN2", target_bir_lowering=False, debug=False)
    ins = {name: nc.dram_tensor(name, shape, dt, kind="ExternalInput").ap()
           for name, (shape, dt) in INPUT_SPECS.items()}
    out = nc.dram_tensor("out", [N, C], F32, kind="ExternalOutput").ap()
    with tile.TileContext(nc) as tc:
        with ExitStack() as ctx:
            kernel_body(ctx, tc, out, ins)
    nc.compile()
    return nc


def _pad_row(row, w):
    a = np.zeros((128, w), np.float32)
    a[0, :] = row
    return a


def make_freq():
    fr = np.linspace(0.0, 1.0, N, dtype=np.float32)
    fb = -((fr - 0.5) ** 2) * 10.0
    return np.ascontiguousarray(fb.reshape(NKT, 128).T).astype(np.float32)


def make_shards(inputs):
    """Full inputs -> list of 8 per-core input dicts."""
    q = np.asarray(inputs["query"], np.float32)
    k = np.asarray(inputs["key"], np.float32)
    v = np.asarray(inputs["value"], np.float32)
    Wq = np.asarray(inputs["Wq"], np.float32); bq = np.asarray(inputs["bq"], np.float32)
    Wk = np.asarray(inputs["Wk"], np.float32); bk = np.asarray(inputs["bk"], np.float32)
    Wv = np.asarray(inputs["Wv"], np.float32); bv = np.asarray(inputs["bv"], np.float32)
    Wp = np.asarray(inputs["Wp"], np.float32); bp = np.asarray(inputs["bp"], np.float32)
    freq = make_freq()
    scale = np.float32(1.0 / np.sqrt(D))

    shards = []
    for c in range(NCORES):
        b, s = c // 2, c % 2
        cs = slice(s * HC, (s + 1) * HC)
        shards.append({
            "xqt": np.ascontiguousarray(q[b].T),
            "xkt": np.ascontiguousarray(k[b].T),
            "xvt": np.ascontiguousarray(v[b].T),
            "wq": np.ascontiguousarray(Wq[:, cs]) * scale,
            "wk": np.ascontiguousarray(Wk[:, cs]),
            "wv": np.ascontiguousarray(Wv[:, cs]),
            "bq": _pad_row(bq[cs] * scale, HC),
            "bk": _pad_row(bk[cs], HC),
            "bv": _pad_row(bv[cs], HC),
            "wp": np.ascontiguousarray(Wp[cs, :]),
            "bp": _pad_row(bp * np.float32(0.5), C),
            "ones": _pad_row(np.ones(512, np.float32), 512),
            "freq": freq,
        })
    return shards


_NC_CACHE = None


def kernel(**inputs):
    global _NC_CACHE
    shards = make_shards(inputs)
    if _NC_CACHE is None:
        _NC_CACHE = build_nc()
    nc = _NC_CACHE
    from concourse import bass_utils
    res = bass_utils.run_bass_kernel_spmd(nc, shards, core_ids=list(range(NCORES)))
    outs = [r["out"] for r in res.results]
    full = np.stack([outs[2 * b] + outs[2 * b + 1] for b in range(B)])
    return full.astype(np.float32)


# revision 13
# speedup vs baseline: 1.4624x; 1.4624x over previous
"""Multihead attention (custom freq-bias) Trainium2 Bass kernel.

Full inputs -> shard across 8 NeuronCores -> SPMD bass kernel -> host combine.

Sharding: core c handles batch b = c//2 and head-half s = c%2 (8 of 16 heads).
Each core computes, for its batch/half:
    qT = (Wq_s^T @ x_q^T) * 1/8 + bq_s   laid out [512 ch, 2048 q]   (scale folded on host)
    kT = Wk_s^T @ x_k^T + bk_s           laid out [512 ch, 2048 k]
    v  = x_v @ Wv_s + bv_s               laid out [2048 k, 512 ch], bf16,
                                          with a ones column per head (denominator trick)
    per head h: scoresT[k, q] = kT_h^T-contracted matmul; exp fused with PSUM
    eviction on ScalarE with per-partition freq-bias; AV matmul with ones column
    produces unnormalized y^T and softmax denominators; deferred normalization;
    out_partial = y @ Wp_s + bp/2  -> host sums the two head-half partials.
"""

import numpy as np
import ml_dtypes

import concourse.bass as bass
import concourse.tile as tile
from concourse import bacc, mybir
from concourse.bass import ts

F32 = mybir.dt.float32
F32R = mybir.dt.float32r
BF16 = mybir.dt.bfloat16
AF = mybir.ActivationFunctionType

B, N, C, H, D = 4, 2048, 1024, 16, 64
NCORES = 8
HC = C // 2          # 512 channels per core (8 heads x 64)
NH = HC // D         # 8 heads per core
NKT = N // 128       # 16 key tiles
VA_W = NH * (D + 1)  # v_aug width: 8 heads x 65 cols


def _r(ap):
    return ap


def kernel_body(ctx, tc, out, ins):
    """Emit the per-core kernel. out: [2048, 1024] f32 DRAM. ins: dict of DRAM APs."""
    nc = tc.nc
    xq, xk, xv = ins["xqt"], ins["xkt"], ins["xvt"]      # [1024, 2048] f32
    wq, wk, wv = ins["wq"], ins["wk"], ins["wv"]          # [1024, 512] f32
    wp = ins["wp"]                                        # [512, 1024] f32
    bq, bk, bv = ins["bq"], ins["bk"], ins["bv"]          # [1, 512] f32
    bp = ins["bp"]                                        # [1, 1024] f32
    freq = ins["freq"]                                    # [128, 16] f32

    singles = ctx.enter_context(tc.tile_pool(name="singles", bufs=1))

    # ---- persistent SBUF residents ----
    qT = [singles.tile([128, N], BF16, name=f"qT{m}") for m in range(4)]
    kT = [singles.tile([128, N], BF16, name=f"kT{m}") for m in range(4)]
    yT = [singles.tile([128, N], BF16, name=f"yT{m}") for m in range(4)]
    vaug = [singles.tile([128, VA_W], BF16, name=f"vaug{i}") for i in range(NKT)]
    wp_sb = [singles.tile([128, C], BF16, name=f"wp{i}") for i in range(4)]
    ones_pad = singles.tile([128, 512], BF16, name="ones_pad")
    bq_pad = singles.tile([128, 512], BF16, name="bq_pad")
    bk_pad = singles.tile([128, 512], BF16, name="bk_pad")
    bv_pad = singles.tile([128, 512], BF16, name="bv_pad")
    bp_pad = singles.tile([128, C], BF16, name="bp_pad")
    freq_sb = singles.tile([128, NKT], F32, name="freq_sb")
    den = singles.tile([32, 512], F32, name="den")
    recip = singles.tile([32, 512], F32, name="recip")
    den_dram = nc.dram_tensor("den_scratch", [32, 512], F32, kind="Internal").ap()

    # constants / bias staging (host-prepared padded tiles; f32r memset is illegal)
    nc.sync.dma_start(out=ones_pad, in_=ins["ones"])
    for pad, src in ((bq_pad, bq), (bk_pad, bk), (bv_pad, bv), (bp_pad, bp)):
        nc.sync.dma_start(out=pad, in_=src)
    nc.sync.dma_start(out=freq_sb, in_=freq)
    for i in range(NKT):
        nc.vector.memset(vaug[i], 1.0)
    for i in range(4):
        nc.sync.dma_start(out=wp_sb[i], in_=wp[ts(i, 128), :])

    # ================= Phase 1: projections =================
    with (
        tc.tile_pool(name="xpool", bufs=12) as xpool,
        tc.tile_pool(name="wpool", bufs=8) as wpool,
        tc.tile_pool(name="psum1", bufs=4, space="PSUM") as psum1,
    ):
        # --- qT / kT: out[ch, q] = W^T @ x^T, ch-major tiles ---
        for (w_dram, x_dram, b_pad, dstT) in (
            (wq, xq, bq_pad, qT),
            (wk, xk, bk_pad, kT),
        ):
            w_sb = []
            for k in range(8):
                t = wpool.tile([128, 512], BF16, tag="w", name=f"w{k}")
                nc.sync.dma_start(out=t, in_=w_dram[ts(k, 128), :])
                w_sb.append(t)
            for nq in range(4):
                x_sb = []
                for k in range(8):
                    t = xpool.tile([128, 512], BF16, tag="x", name=f"x{k}")
                    nc.sync.dma_start(out=t, in_=x_dram[ts(k, 128), ts(nq, 512)])
                    x_sb.append(t)
                for m in range(4):
                    ps = psum1.tile([128, 512], F32, tag="ps1", name="ps_qk")
                    nc.tensor.matmul(ps, _r(b_pad[:, ts(m, 128)]), _r(ones_pad),
                                     start=True, stop=False)
                    for k in range(8):
                        nc.tensor.matmul(ps, _r(w_sb[k][:, ts(m, 128)]), _r(x_sb[k]),
                                         start=False, stop=(k == 7))
                    nc.vector.tensor_copy(dstT[m][:, ts(nq, 512)], ps)

        # --- v: out[n, ch] = x @ Wv, scattered into v_aug (bf16, ones cols kept) ---
        wv_sb = []
        for k in range(8):
            t = wpool.tile([128, 512], BF16, tag="w", name=f"wv{k}")
            nc.sync.dma_start(out=t, in_=wv[ts(k, 128), :])
            wv_sb.append(t)
        for ntg in range(4):
            xv_sb = []
            for k in range(8):
                t = xpool.tile([128, 512], BF16, tag="x", name=f"xv{k}")
                nc.sync.dma_start(out=t, in_=xv[ts(k, 128), ts(ntg, 512)])
                xv_sb.append(t)
            for ntl in range(4):
                nt = ntg * 4 + ntl
                ps = psum1.tile([128, 512], F32, tag="ps1", name="ps_v")
                nc.tensor.matmul(ps, _r(ones_pad[:, 0:128]), _r(bv_pad),
                                 start=True, stop=False)
                for k in range(8):
                    nc.tensor.matmul(ps, _r(xv_sb[k][:, ts(ntl, 128)]), _r(wv_sb[k]),
                                     start=False, stop=(k == 7))
                va = vaug[nt].rearrange("p (h c) -> p h c", c=D + 1)
                nc.vector.tensor_copy(va[:, :, 0:D],
                                      ps.rearrange("p (h c) -> p h c", c=D))

    # ================= Phase 2: attention =================
    with (
        tc.tile_pool(name="epool", bufs=11) as epool,
        tc.tile_pool(name="dpool", bufs=4) as dpool,
        tc.tile_pool(name="psS", bufs=2, space="PSUM") as psS,
        tc.tile_pool(name="psA", bufs=1, space="PSUM") as psA_pool,
    ):
        for h in range(NH):
            pair, po = h // 2, (h % 2) * 64
            psA = [psA_pool.tile([128, 512], F32, tag=f"psA{qc}", name=f"psA{qc}")
                   for qc in range(4)]
            ets = [None] * NKT
            for half in range(2):
                for kt8 in range(8):
                    kt = half * 8 + kt8
                    et = epool.tile([128, N], BF16, tag="exp", name="et")
                    for qp in range(2):
                        pss = psS.tile([128, 1024], F32, tag="pss", name="pss")
                        for j in range(2):
                            qc = qp * 2 + j
                            nc.tensor.matmul(
                                pss[:, ts(j, 512)],
                                _r(kT[pair][po:po + 64, ts(kt, 128)]),
                                _r(qT[pair][po:po + 64, ts(qc, 512)]),
                                start=True, stop=True)
                        nc.scalar.activation(out=et[:, ts(qp, 1024)], in_=pss,
                                             func=AF.Exp,
                                             bias=freq_sb[:, kt:kt + 1], scale=1.0)
                    ets[kt] = et
                for qc in range(4):
                    for kt8 in range(8):
                        kt = half * 8 + kt8
                        nc.tensor.matmul(
                            psA[qc][0:D + 1, :],
                            vaug[kt][:, h * (D + 1):(h + 1) * (D + 1)],
                            ets[kt][:, ts(qc, 512)],
                            start=(kt == 0), stop=(kt == NKT - 1),
                            skip_group_check=True)
            for qc in range(4):
                nc.vector.tensor_copy(yT[pair][po:po + 64, ts(qc, 512)],
                                      psA[qc][0:D, :])
                dt = dpool.tile([1, 512], F32, tag="dt", name="dt")
                nc.vector.tensor_copy(dt, psA[qc][D:D + 1, :])
                r = h * 4 + qc
                nc.sync.dma_start(out=den_dram[r:r + 1, :], in_=dt)

    # ================= Phase 3: normalize + output projection =================
    nc.sync.dma_start(out=den, in_=den_dram)
    nc.vector.reciprocal(recip, den)
    recip_dram = nc.dram_tensor("recip_scratch", [32, 512], F32, kind="Internal").ap()
    nc.sync.dma_start(out=recip_dram, in_=recip)
    with (
        tc.tile_pool(name="rpool", bufs=4) as rpool,
        tc.tile_pool(name="opool", bufs=4) as opool,
        tc.tile_pool(name="psO", bufs=4, space="PSUM") as psO,
    ):
        for pair in range(4):
            for qc in range(4):
                r = rpool.tile([128, 512], F32, tag="rbc", name="rbc")
                for hh in range(2):
                    src = recip_dram[(pair * 2 + hh) * 4 + qc:(pair * 2 + hh) * 4 + qc + 1, :]
                    bcast = bass.AP(tensor=src.tensor, offset=src.offset,
                                    ap=[[0, 64]] + list(src.ap)[1:])
                    nc.sync.dma_start(out=r[hh * 64:(hh + 1) * 64, :], in_=bcast)
                ysl = yT[pair][:, ts(qc, 512)]
                nc.vector.tensor_mul(ysl, ysl, r)
        for m in range(16):
            for n2 in range(2):
                ps = psO.tile([128, 512], F32, tag="psO", name="psO")
                nc.tensor.matmul(ps, _r(ones_pad[:, 0:128]),
                                 _r(bp_pad[:, ts(n2, 512)]), start=True, stop=False)
                for kp in range(4):
                    nc.tensor.matmul(ps, _r(yT[kp][:, ts(m, 128)]),
                                     _r(wp_sb[kp][:, ts(n2, 512)]),
                                     start=False, stop=(kp == 3))
                ot = opool.tile([128, 512], F32, tag="ot", name="ot")
                nc.scalar.copy(ot, ps)
                nc.sync.dma_start(out=out[ts(m, 128), ts(n2, 512)], in_=ot)


INPUT_SPECS = {
    "xqt": ([C, N], BF16), "xkt": ([C, N], BF16), "xvt": ([C, N], BF16),
    "wq": ([C, HC], BF16), "wk": ([C, HC], BF16), "wv": ([C, HC], BF16),
    "bq": ([128, HC], BF16), "bk": ([128, HC], BF16), "bv": ([128, HC], BF16),
    "wp": ([HC, C], BF16), "bp": ([128, C], BF16),
    "ones": ([128, 512], BF16),
    "freq": ([128, NKT], F32),
}


def build_nc():
    from contextlib import ExitStack
    nc = bacc.Bacc("TRN2", target_bir_lowering=False, debug=False)
    ins = {name: nc.dram_tensor(name, shape, dt, kind="ExternalInput").ap()
           for name, (shape, dt) in INPUT_SPECS.items()}
    out = nc.dram_tensor("out", [N, C], F32, kind="ExternalOutput").ap()
    with tile.TileContext(nc) as tc:
        with ExitStack() as ctx:
            kernel_body(ctx, tc, out, ins)
    nc.compile()
    return nc


def _pad_row(row, w):
    a = np.zeros((128, w), np.float32)
    a[0, :] = row
    return a


def make_freq():
    fr = np.linspace(0.0, 1.0, N, dtype=np.float32)
    fb = -((fr - 0.5) ** 2) * 10.0
    return np.ascontiguousarray(fb.reshape(NKT, 128).T).astype(np.float32)


def make_shards(inputs):
    """Full inputs -> list of 8 per-core input dicts."""
    q = np.asarray(inputs["query"], np.float32)
    k = np.asarray(inputs["key"], np.float32)
    v = np.asarray(inputs["value"], np.float32)
    Wq = np.asarray(inputs["Wq"], np.float32); bq = np.asarray(inputs["bq"], np.float32)
    Wk = np.asarray(inputs["Wk"], np.float32); bk = np.asarray(inputs["bk"], np.float32)
    Wv = np.asarray(inputs["Wv"], np.float32); bv = np.asarray(inputs["bv"], np.float32)
    Wp = np.asarray(inputs["Wp"], np.float32); bp = np.asarray(inputs["bp"], np.float32)
    freq = make_freq()
    scale = np.float32(1.0 / np.sqrt(D))

    shards = []
    for c in range(NCORES):
        b, s = c // 2, c % 2
        cs = slice(s * HC, (s + 1) * HC)
        shards.append({
            "xqt": np.ascontiguousarray(q[b].T),
            "xkt": np.ascontiguousarray(k[b].T),
            "xvt": np.ascontiguousarray(v[b].T),
            "wq": np.ascontiguousarray(Wq[:, cs]) * scale,
            "wk": np.ascontiguousarray(Wk[:, cs]),
            "wv": np.ascontiguousarray(Wv[:, cs]),
            "bq": _pad_row(bq[cs] * scale, HC),
            "bk": _pad_row(bk[cs], HC),
            "bv": _pad_row(bv[cs], HC),
            "wp": np.ascontiguousarray(Wp[cs, :]),
            "bp": _pad_row(bp * np.float32(0.5), C),
            "ones": _pad_row(np.ones(512, np.float32), 512),
            "freq": freq,
        })
        s_ = shards[-1]
        for kk in ("xqt","xkt","xvt","wq","wk","wv","bq","bk","bv","wp","bp","ones"):
            s_[kk] = np.asarray(s_[kk]).astype(ml_dtypes.bfloat16)
    return shards


_NC_CACHE = None


def kernel(**inputs):
    global _NC_CACHE
    shards = make_shards(inputs)
    if _NC_CACHE is None:
        _NC_CACHE = build_nc()
    nc = _NC_CACHE
    from concourse import bass_utils
    res = bass_utils.run_bass_kernel_spmd(nc, shards, core_ids=list(range(NCORES)))
    outs = [r["out"] for r in res.results]
    full = np.stack([outs[2 * b] + outs[2 * b + 1] for b in range(B)])
    return full.astype(np.float32)
